# revision 8
# baseline (speedup 1.0000x reference)
"""MI-estimator loss kernel v5.

Device per core (1024 rows): L1 (bf16) -> relu -> L2 (bf16) -> raw psum
copied to SBUF bf16 -> SWDGE scatter-add (prepped early, triggered when
copies land) into runtime-pre-zeroed DRAM outputs.  Host does
bias/tanh/exp/loss in fp64.

Inputs:
  xk  (128, 2048) bf16 : col blocks [n0k0 | n0k1 | n1k0 | n1k1], each
                         [p, n] = x[nbase+n, 128k+p]
  wgt (128, 1288) bf16 : w1-lv [m0k0|m1k0|m0k1|m1k1] (0:512) |
                         b1 f32-bitcast cols (512:520) |
                         w1-mu (520:1032) | w2 [mu-m0|mu-m1|lv-m0|lv-m1]
Outputs:
  olv, omu (128, 512) bf16 : [p, c] -> d = p%64, n = 512*(p//64)+c
"""

import sys

import numpy as np

try:
    import concourse.bass  # noqa: F401
except ImportError:
    for p in ("/opt/trn_rl_repo", "/root/.axon_site/_ro/trn_rl_repo"):
        if p not in sys.path:
            sys.path.insert(0, p)

N, DX, DY, H = 8192, 256, 64, 256
NCORES = 8
NLOC = N // NCORES  # 1024
NH = NLOC // 2  # 512

WCOLS = 1288
N_WARMUP = 227

_CACHE = {}


def _build_nc():
    import concourse.bass as bass
    import concourse.mybir as mybir
    import concourse.tile as tile
    from concourse import bacc
    from concourse.bass import _add_dep_helper

    f32 = mybir.dt.float32
    f32r = mybir.dt.float32r
    bf16 = mybir.dt.bfloat16
    i16 = mybir.dt.int16
    AF = mybir.ActivationFunctionType
    ALU = mybir.AluOpType

    nc = bacc.Bacc(
        trn_type="TRN2",
        target_bir_lowering=False,
        debug=False,
        num_devices=NCORES,
        num_swdge_queues=3,
    )

    xk = nc.dram_tensor("xk", (128, 2 * NLOC), bf16, kind="ExternalInput").ap()
    wgt = nc.dram_tensor("wgt", (128, WCOLS), bf16, kind="ExternalInput").ap()
    olv = nc.dram_tensor("olv", (128, NH), bf16, kind="ExternalOutput").ap()
    omu = nc.dram_tensor("omu", (128, NH), bf16, kind="ExternalOutput").ap()
    odram = {"lv": olv, "mu": omu}
    QNUM = {"lv": 1, "mu": 2}

    sem_out = {"lv": nc.alloc_semaphore("sc_lv"),
               "mu": nc.alloc_semaphore("sc_mu")}

    with tile.TileContext(nc) as tc:
        with (
            tc.tile_pool(name="const", bufs=1) as const,
            tc.tile_pool(name="xp", bufs=1) as xp,
            tc.tile_pool(name="hp", bufs=1) as hp,
            tc.tile_pool(name="op", bufs=1) as op,
            tc.tile_pool(name="psp", bufs=1, space="PSUM") as psp,
        ):
            # ---- SBUF tiles ------------------------------------------
            x_sb = xp.tile([128, 2 * NLOC], bf16, tag="x")
            w_sb = const.tile([128, WCOLS], bf16, tag="w")
            idx_sb = const.tile([16, 8], i16, tag="idx")
            warm = const.tile([128, 8], f32, tag="warm")
            scratch = const.tile([128, 1], f32, tag="scratch")
            hT = {}
            for head in ("lv", "mu"):
                for m in range(2):
                    t = hp.tile([128, NLOC], bf16, tag=f"h{head}{m}")
                    hT[(head, m)] = t
            o_sb = {}
            for h in ("lv", "mu"):
                t = op.tile([128, NH], bf16, tag=f"o{h}")
                o_sb[h] = t

            # ---- Pool sequence (pinned order) ------------------------
            _prev_pool = [None]

            def pool_pin(inst):
                if _prev_pool[0] is not None:
                    _add_dep_helper(inst.ins, _prev_pool[0].ins, sync=False,
                                    reason="pin Pool order")
                _prev_pool[0] = inst
                return inst

            pool_pin(nc.gpsimd.memset(warm, 0.0))
            # idx[p, c] = 16*c + p  (identity scatter indices, wrapped)
            pool_pin(nc.gpsimd.iota(idx_sb, [[16, 8]], base=0,
                                    channel_multiplier=1))
            # n1 x halves via Pool SWDGE (bypasses the shared HWDGE)
            pool_pin(nc.gpsimd.dma_start(out=x_sb[:, 2 * NH:3 * NH],
                                         in_=xk[:, 2 * NH:3 * NH]))
            pool_pin(nc.gpsimd.dma_start(out=x_sb[:, 3 * NH:4 * NH],
                                         in_=xk[:, 3 * NH:4 * NH]))

            # ---- input DMAs (SP engine, HWDGE) -----------------------
            for lo, hi, src, dst in [
                (0, 520, wgt, w_sb),          # w1-lv + biases
                (0, 512, xk, x_sb),           # x n0 k0
                (512, 1024, xk, x_sb),        # x n0 k1
                (520, 1032, wgt, w_sb),       # w1-mu
                (1032, 1288, wgt, w_sb),      # w2
            ]:
                nc.sync.dma_start(out=dst[:, lo:hi], in_=src[:, lo:hi])

            # ---- ACT table preload -----------------------------------
            nc.scalar.activation(out=scratch, in_=warm[:, 0:1], func=AF.Relu)

            # accessors
            def w1_ap(head, k, m):
                off = (0 if head == "lv" else 520) + 256 * k + 128 * m
                return w_sb[:, off:off + 128]

            def w2_ap(head, m):
                off = 1032 + (0 if head == "mu" else 128) + 64 * m
                return w_sb[:, off:off + 64]

            def b1_ap(head, m):
                c = (0 if head == "mu" else 2) + m
                return w_sb[0:128, 512 + 2 * c:514 + 2 * c].bitcast(f32)

            def x_ap(nh, k):
                blk = 2 * nh + k
                return x_sb[:, NH * blk:NH * (blk + 1)]

            # ---- PSUM ------------------------------------------------
            ps = psp.tile([128, 4 * NLOC], f32, tag="ps")
            L1_BASE = {("lv", 0): 0, ("lv", 1): NLOC,
                       ("mu", 0): 2 * NLOC, ("mu", 1): 3 * NLOC}
            # transposed-L2 regions: per (head, nhalf) one 256-col span in
            # a bank freed by an early lv relu; chunk c occupies 64 cols
            L2T_BASE = {("lv", 0): 0, ("lv", 1): NH,
                        ("mu", 0): NLOC, ("mu", 1): NLOC + NH}

            _prev_mm = [None]

            def mm(out_ap, lhsT, rhs, start, stop):
                m = nc.tensor.matmul(out_ap, lhsT=lhsT, rhs=rhs, start=start,
                                     stop=stop)
                if _prev_mm[0] is not None:
                    _add_dep_helper(m.ins, _prev_mm[0].ins, sync=False,
                                    reason="pin PE order")
                _prev_mm[0] = m
                return m

            warm_r = warm.bitcast(f32r)
            for _ in range(N_WARMUP):
                mm(ps[0:8, 0:8], warm_r[:, 0:8], warm_r[:, 0:8],
                   True, True)

            def l1(head, m, k, nh):
                base = L1_BASE[(head, m)]
                mm(ps[:, base + nh * NH:base + (nh + 1) * NH],
                   w1_ap(head, k, m), x_ap(nh, k), k == 0, k == 1)

            def l2t(head, c, morder=(0, 1)):
                # chunk c covers n rows 128c:128(c+1); out [128, 64] psum
                base = L2T_BASE[(head, c // 4)] + 64 * (c % 4)
                for i, m in enumerate(morder):
                    mm(ps[:, base:base + 64],
                       hT[(head, m)][:, 128 * c:128 * (c + 1)],
                       w2_ap(head, m),
                       i == 0, i == 1)

            def relu(head, m, nh, eng, clo=0, chi=NH):
                base = L1_BASE[(head, m)]
                sl_ps = ps[:, base + nh * NH + clo:base + nh * NH + chi]
                sl_h = hT[(head, m)][:, nh * NH + clo:nh * NH + chi]
                if eng == "act":
                    return nc.scalar.activation(out=sl_h, in_=sl_ps,
                                                func=AF.Relu,
                                                bias=b1_ap(head, m))
                if eng == "pool":
                    return pool_pin(nc.gpsimd.tensor_scalar(
                        out=sl_h, in0=sl_ps, scalar1=b1_ap(head, m),
                        scalar2=0.0, op0=ALU.add, op1=ALU.max))
                return nc.vector.tensor_scalar(
                    out=sl_h, in0=sl_ps, scalar1=b1_ap(head, m),
                    scalar2=0.0, op0=ALU.add, op1=ALU.max)

            def copy_out(head, ng, eng):
                # one [128, 256] copy per (head, n-half): chunks 4ng..4ng+3
                base = L2T_BASE[(head, ng)]
                src = ps[:, base:base + 256]
                dst = o_sb[head][:, 256 * ng:256 * (ng + 1)]
                if eng == "act":
                    return nc.scalar.activation(out=dst, in_=src,
                                                func=AF.Copy)
                return nc.vector.tensor_scalar(
                    out=dst, in0=src, scalar1=0.0, scalar2=0.0,
                    op0=ALU.add, op1=ALU.bypass)

            def prep_scatter(h):
                return nc.gpsimd.dma_scatter_add(
                    odram[h][:],
                    o_sb[h][:, None, :],
                    idx_sb[:],
                    128, 128, NH,
                    prepare_only=True,
                    sem=sem_out[h],
                    queue_num=QNUM[h],
                )

            # ---- schedule (statements in expected execution order) ---
            l1("lv", 0, 0, 0)
            l1("lv", 1, 0, 0)
            l1("lv", 0, 0, 1)
            l1("lv", 1, 0, 1)
            l1("lv", 0, 1, 0)
            l1("lv", 1, 1, 0)
            relu("lv", 0, 0, "act")
            relu("lv", 1, 0, "dve")
            l1("lv", 0, 1, 1)
            l1("lv", 1, 1, 1)
            relu("lv", 0, 1, "act")
            relu("lv", 1, 1, "dve")
            l1("mu", 0, 0, 0)
            l1("mu", 1, 0, 0)
            l1("mu", 0, 1, 0)
            l1("mu", 1, 1, 0)
            relu("mu", 0, 0, "act")
            relu("mu", 1, 0, "dve")
            l1("mu", 0, 0, 1)
            l1("mu", 1, 0, 1)
            l1("mu", 1, 1, 1)          # m1 first: its relu gates L2-mu-n1
            l1("mu", 0, 1, 1)
            relu("mu", 1, 1, "dve")
            relu("mu", 0, 1, "act")
            for c in range(4, 8):
                l2t("lv", c)
            copy_out("lv", 1, "dve")
            for c in range(4):
                l2t("lv", c)
            copy_out("lv", 0, "act")
            for c in range(4):
                l2t("mu", c)
            copy_out("mu", 0, "act")
            for c in range(4, 8):
                l2t("mu", c, morder=(1, 0))
            copy_out("mu", 1, "dve")

            nc.sync.dma_start(out=olv, in_=o_sb["lv"])
            nc.sync.dma_start(out=omu, in_=o_sb["mu"])

    nc.compile()
    return nc


def _get_nc():
    if "nc" not in _CACHE:
        _CACHE["nc"] = _build_nc()
    return _CACHE["nc"]


def _pack_weights(inputs, bfdt):
    wgt = np.zeros((128, WCOLS), dtype=np.float32)
    for head, w1 in (("lv", inputs["lv_w1"]), ("mu", inputs["mu_w1"])):
        hoff = 0 if head == "lv" else 520
        for k in range(2):
            for m in range(2):
                wgt[:, hoff + 256 * k + 128 * m:
                    hoff + 256 * k + 128 * m + 128] = \
                    w1[128 * k:128 * (k + 1), 128 * m:128 * (m + 1)]
    for head, w2 in (("mu", inputs["mu_w2"]), ("lv", inputs["lv_w2"])):
        hoff = 1032 + (0 if head == "mu" else 128)
        for m in range(2):
            wgt[:, hoff + 64 * m:hoff + 64 * (m + 1)] = \
                w2[128 * m:128 * (m + 1), :]
    wb = np.ascontiguousarray(wgt.astype(bfdt))
    # biases ride as raw f32 bit patterns in bf16 cols 512:520
    bias = np.zeros((128, 4), dtype=np.float32)
    bias[:, 0] = inputs["mu_b1"][:128]
    bias[:, 1] = inputs["mu_b1"][128:]
    bias[:, 2] = inputs["lv_b1"][:128]
    bias[:, 3] = inputs["lv_b1"][128:]
    bits = bias.view(np.uint32)
    wb_u16 = wb.view(np.uint16)
    wb_u16[:, 512:520:2] = (bits & 0xFFFF).astype(np.uint16)
    wb_u16[:, 513:520:2] = (bits >> 16).astype(np.uint16)
    return wb


def kernel(emb_x, emb_y, mu_w1, mu_b1, mu_w2, mu_b2, lv_w1, lv_b1, lv_w2, lv_b2):
    import ml_dtypes
    from concourse.bass_utils import run_bass_kernel_spmd

    bfdt = ml_dtypes.bfloat16
    inputs = {
        "emb_x": np.asarray(emb_x, np.float32),
        "emb_y": np.asarray(emb_y, np.float32),
        "mu_w1": np.asarray(mu_w1, np.float32),
        "mu_b1": np.asarray(mu_b1, np.float32),
        "mu_w2": np.asarray(mu_w2, np.float32),
        "mu_b2": np.asarray(mu_b2, np.float32),
        "lv_w1": np.asarray(lv_w1, np.float32),
        "lv_b1": np.asarray(lv_b1, np.float32),
        "lv_w2": np.asarray(lv_w2, np.float32),
        "lv_b2": np.asarray(lv_b2, np.float32),
    }
    wgt = _pack_weights(inputs, bfdt)

    in_maps = []
    for c in range(NCORES):
        xsh = inputs["emb_x"][c * NLOC:(c + 1) * NLOC]  # (1024, 256)
        xT = np.ascontiguousarray(xsh.T)
        blocks = [xT[0:128, 0:512], xT[128:256, 0:512],
                  xT[0:128, 512:1024], xT[128:256, 512:1024]]
        xkc = np.concatenate(blocks, axis=1)  # (128, 2048)
        in_maps.append({"xk": np.ascontiguousarray(xkc.astype(bfdt)),
                        "wgt": wgt})

    nc = _get_nc()
    res = run_bass_kernel_spmd(nc, in_maps, list(range(NCORES)))

    # ---- host-side finish (fp64) --------------------------------------
    y = inputs["emb_y"].astype(np.float64)
    ybar = y.mean(axis=0)
    y2bar = (y ** 2).mean(axis=0)
    mu_b2_ = inputs["mu_b2"].astype(np.float64)
    lv_b2_ = inputs["lv_b2"].astype(np.float64)

    total = 0.0
    for c in range(NCORES):
        raw_lv = np.asarray(res.results[c]["olv"]).astype(np.float64)
        raw_mu = np.asarray(res.results[c]["omu"]).astype(np.float64)
        lv = raw_lv.reshape(128, 8, 64).transpose(1, 0, 2).reshape(1024, 64).T
        mu = raw_mu.reshape(128, 8, 64).transpose(1, 0, 2).reshape(1024, 64).T
        mu = mu + mu_b2_[:, None]
        logvar = np.tanh(lv + lv_b2_[:, None])
        iv = np.exp(-logvar)
        yT = y[c * NLOC:(c + 1) * NLOC].T
        t = (yT ** 2 - 2.0 * mu * yT + 2.0 * mu * ybar[:, None]
             - y2bar[:, None]) * iv
        total += t.sum()

    loss = -0.5 / N * total
    return np.float32(loss)
